# revision 2
# baseline (speedup 1.0000x reference)
"""Trainium2 Bass kernel v2 for the segment_reduce loss (bf16 streaming).

Key insight from microbenchmarks: the DMA bottleneck is the per-core SBUF
write fabric (~435 GB/s), not HBM reads. A SWDGE f32->bf16 cast-DMA halves
SBUF-side bytes: the 5-array stream drops from ~86us to ~45us per core when
transferred as [128, 4096] 2MB-source chunks. Tolerance is 2e-2; bf16 on the
data path costs ~1e-4 relative error (argmin tie-flips are zero-mean noise).

Per core (S=8192 curves, NSH=2M elements per array):
  - M=4 chunks of [P=128, F=4096] bf16 (J=16 curves per partition-row).
  - GPSIMD: issues the 5 cast-DMAs per chunk (SWDGE) + Acj = Ac - Aj, d = An - Ar.
  - ACT: A = |Acj|, Square(d) accum -> mse, Relu(-Ap) accum -> apn, end copies.
  - DVE: G = 1.1*Aj - Ap (STT), 3 segmented 3D reduces (min/sum/sum), 16
    argmin-select STTs (is_equal -> mult -> accum) per chunk.
  - Epilogue on [128, 64] f32 per-curve blocks, one [P, 288] f32 acc DMA out.
  - Host folds Ci-end gather, correlation/sign terms, and the 8 cores' blocks
    in float64 (host terms only touch C-length or O(4) inputs).
"""

import os
import sys

import numpy as np

sys.path.insert(0, "/opt/trn_rl_repo")

import concourse.bass as bass
import concourse.bacc as bacc
import concourse.tile as tile
from concourse import mybir
from concourse.bass_utils import run_bass_kernel_spmd
from contextlib import ExitStack

NCORES = 8
C = 65536
L = 256
N = C * L
S = C // NCORES          # curves per core
NSH = S * L              # elements per core per big array
P = 128                  # partitions
F = 4096                 # elements per partition per chunk
J = F // L               # curves per partition per chunk (16)
M = NSH // (P * F)       # chunks per core (4)
NCOL = M * J             # per-curve accumulator columns (64)

KELVIN = 273.15
FIT_AP_CI = 500.0
TARGET_R = 0.7

f32 = mybir.dt.float32
bf16 = mybir.dt.bfloat16

# accumulator block column layout (acc is [P, ACCW] f32)
MSE0 = 0             # [M] sum (An-Ar)^2          (d-mode)
MSA0 = MSE0 + M      # [M] sum An^2               (squares-mode)
MSB0 = MSA0 + M      # [M] sum Ar^2               (squares-mode)
MSC0 = MSB0 + M      # [M] sum An*Ar              (squares-mode)
APN0 = MSC0 + M      # [M] sum relu(-Ap) (or -sum min(Ap,0) pre-negate)
P30 = APN0 + M       # [NCOL] relu(3*gint) per curve
LS0 = P30 + NCOL     # [NCOL] w*(relu(8-ls_Aj)+relu(8-ls_Ac))
E10 = LS0 + NCOL     # [NCOL] relu(Ap_end-Aj_end)*fitw
E20 = E10 + NCOL     # [NCOL] relu(Aj_end-Ac_end)
ACCW = E20 + NCOL    # 276 -> pad to 288
ACCW = 288

VARIANT = dict(
    inp_bufs=2,
    wrk_bufs=2,
    acj_eng="gpsimd",     # gpsimd | vector
    d_mode="vector",      # gpsimd | vector | squares
    a_eng="scalar",       # scalar (ACT Abs) | vector (DVE STT max(-x,x))
    apn_eng="scalar",     # scalar (Relu scale=-1 accum) | vector (TS min accum, negated later)
    ends_eng="vector",    # scalar | vector | gpsimd
    epi_on_pool=False,    # epilogue tensor_tensor ops on GPSIMD (else DVE)
    dma_order=("Ac", "Aj", "Ap", "An", "Ar"),
    chunked_epi=False,
    acc_bufs=2,
    sel_mode="stt",       # stt (J small STTs/chunk) | scan | bcast (stride-0 TT)
    g_mode="act",         # stt (DVE STT 1.1*Aj-Ap) | act (ACT scale + DVE TT sub)
    reduce_tree=2,        # levels of TT-halving before each segmented reduce
    issue_ahead=False,    # enqueue chunk m+1's DMAs before chunk m's compute
    epi_mode="mixed",     # mixed (TT + ACT relu) | dve (all-DVE, TS-fused relu)
    drop=(),              # ablation: any of sel,reduces,ends,g,d,apn,a,acj,epi
    max_stage=99,         # debug: only emit compute stages <= this
)


def _build_kernel(reps=None, variant=None):
    OP = mybir.AluOpType
    AF = mybir.ActivationFunctionType
    AX = mybir.AxisListType
    v = dict(VARIANT)
    if variant:
        v.update(variant)

    nc = bacc.Bacc("TRN2", target_bir_lowering=False, debug=False, num_devices=NCORES)
    big = {
        nm: nc.declare_dram_parameter(nm, [NSH], f32, isOutput=False)
        for nm in ("An", "Ar", "Ac", "Aj", "Ap")
    }
    wdev = nc.declare_dram_parameter("wdev", [P, NCOL], f32, isOutput=False)
    fitw = nc.declare_dram_parameter("fitw", [P, NCOL], f32, isOutput=False)
    acc = nc.declare_dram_parameter("acc", [P, ACCW], f32, isOutput=True)

    with ExitStack() as ctx:
        tc = ctx.enter_context(tile.TileContext(nc))
        inp = ctx.enter_context(tc.tile_pool(name="inp", bufs=v["inp_bufs"]))
        wrk = ctx.enter_context(tc.tile_pool(name="wrk", bufs=v["wrk_bufs"]))
        per = ctx.enter_context(tc.tile_pool(name="per", bufs=1))
        apool = ctx.enter_context(tc.tile_pool(name="apool", bufs=v["acc_bufs"]))

        wT = per.tile([P, NCOL], f32, tag="wT")
        fT = per.tile([P, NCOL], f32, tag="fT")
        junkD = per.tile([P, L], bf16, tag="junkD")    # sel STT outs
        if v["reduce_tree"]:
            H1 = per.tile([P, F // 2], bf16, tag="H1")   # DVE-serial scratch
            H2 = per.tile([P, F // 4], bf16, tag="H2")
        if v["sel_mode"] == "scan":
            Bt = per.tile([P, F], bf16, tag="Bt")      # 0 at curve starts, 1 else
            zS = per.tile([P, F], bf16, tag="zS")      # mn at starts, 0 else
            mnF = per.tile([P, F], bf16, tag="mnF")    # broadcast min (DVE-serial)
            gm = per.tile([P, F], bf16, tag="gm")      # mask tile (DVE-serial)
            nc.vector.memset(Bt, 1.0)
            nc.vector.memset(Bt.rearrange("p (j l) -> p j l", l=L)[:, :, 0:1], 0.0)
            nc.vector.memset(zS, 0.0)
        junkA = per.tile([P, F], bf16, tag="junkA")    # ACT accum outs
        junkV = (per.tile([P, F], bf16, tag="junkV")
                 if (v["apn_eng"] != "scalar" or v["d_mode"] == "squares")
                 else junkA)  # DVE accum outs (only needed in those modes)
        b8 = per.tile([P, 1], f32, tag="b8")
        nc.vector.memset(b8, 8.0)

        nc.sync.dma_start(out=wT, in_=wdev[:])
        nc.sync.dma_start(out=fT, in_=fitw[:])

        def body():
            # per-rep accumulator tiles: bufs=2 so rep r+1's first writes don't
            # wait on rep r's epilogue/acc-DMA reads
            accT = apool.tile([P, ACCW], f32, tag="accT")
            mnB = apool.tile([P, NCOL], bf16, tag="mnB")
            sAcj = apool.tile([P, NCOL], f32, tag="sAcj")
            sAbs = apool.tile([P, NCOL], f32, tag="sAbs")
            gint = apool.tile([P, NCOL], f32, tag="gint")
            eAp = apool.tile([P, NCOL], f32, tag="eAp")
            eAj = apool.tile([P, NCOL], f32, tag="eAj")
            eAc = apool.tile([P, NCOL], f32, tag="eAc")
            t1 = apool.tile([P, NCOL], f32, tag="t1")
            t2 = apool.tile([P, NCOL], f32, tag="t2")
            r1 = apool.tile([P, NCOL], f32, tag="r1")
            r2 = apool.tile([P, NCOL], f32, tag="r2")
            if v["max_stage"] < 10:
                nc.vector.memset(accT, 0.0)
            else:
                # zero the mse-mode columns not written in this d_mode + padding
                if v["d_mode"] in ("gpsimd", "vector"):
                    nc.vector.memset(accT[:, MSA0:APN0], 0.0)
                else:
                    nc.vector.memset(accT[:, MSE0:MSA0], 0.0)
                nc.vector.memset(accT[:, E20 + NCOL : ACCW], 0.0)
            tiles = {}

            def issue(m):
                tiles[m] = {}
                for nm in v["dma_order"]:
                    tl = inp.tile([P, F], bf16, tag=nm, name=f"in_{nm}_{m}")
                    tiles[m][nm] = tl
                    src = big[nm][:].rearrange("(m p f) -> m p f", m=M, p=P, f=F)[m]
                    nc.gpsimd.dma_start(out=tl, in_=src)

            if v["issue_ahead"]:
                issue(0)
            for m in range(M):
                if v["issue_ahead"]:
                    if m + 1 < M:
                        issue(m + 1)
                else:
                    issue(m)
                t = tiles.pop(m)

                cols = slice(m * J, (m + 1) * J)

                if v["max_stage"] < 2:
                    continue
                if "acj" in v["drop"]:
                    continue
                Acj = wrk.tile([P, F], bf16, tag="Acj")
                acj_eng = nc.gpsimd if v["acj_eng"] == "gpsimd" else nc.vector
                acj_eng.tensor_tensor(out=Acj, in0=t["Ac"], in1=t["Aj"], op=OP.subtract)

                # --- mse ---
                if v["max_stage"] < 3:
                    continue
                if "d" in v["drop"]:
                    pass
                elif v["d_mode"] in ("gpsimd", "vector"):
                    d = wrk.tile([P, F], bf16, tag="d")
                    d_eng = nc.gpsimd if v["d_mode"] == "gpsimd" else nc.vector
                    d_eng.tensor_tensor(out=d, in0=t["An"], in1=t["Ar"], op=OP.subtract)
                    nc.scalar.activation(
                        out=junkA, in_=d, func=AF.Square,
                        accum_out=accT[:, MSE0 + m : MSE0 + m + 1],
                    )
                else:  # squares: sum An^2 + sum Ar^2 - 2 sum An*Ar on host
                    nc.scalar.activation(
                        out=junkA, in_=t["An"], func=AF.Square,
                        accum_out=accT[:, MSA0 + m : MSA0 + m + 1],
                    )
                    nc.scalar.activation(
                        out=junkA, in_=t["Ar"], func=AF.Square,
                        accum_out=accT[:, MSB0 + m : MSB0 + m + 1],
                    )
                    nc.vector.tensor_tensor_reduce(
                        out=junkV, in0=t["An"], in1=t["Ar"], scale=1.0, scalar=0.0,
                        op0=OP.mult, op1=OP.add,
                        accum_out=accT[:, MSC0 + m : MSC0 + m + 1],
                    )

                # --- apn: sum relu(-Ap) ---
                if v["max_stage"] < 4:
                    continue
                if "apn" in v["drop"]:
                    pass
                elif v["apn_eng"] == "scalar":
                    nc.scalar.activation(
                        out=junkA, in_=t["Ap"], func=AF.Relu, scale=-1.0,
                        accum_out=accT[:, APN0 + m : APN0 + m + 1],
                    )
                else:
                    # accum = sum(min(Ap,0)) = -sum relu(-Ap); negated in epilogue
                    nc.vector.tensor_scalar(
                        out=junkV, in0=t["Ap"], scalar1=0.0, scalar2=None,
                        op0=OP.min, op1=OP.add,
                        accum_out=accT[:, APN0 + m : APN0 + m + 1],
                    )

                # --- A = |Acj| ---
                if v["max_stage"] < 5:
                    continue
                if "a" in v["drop"]:
                    continue
                A = wrk.tile([P, F], bf16, tag="A")
                if v["a_eng"] == "scalar":
                    nc.scalar.activation(out=A, in_=Acj, func=AF.Abs)
                else:
                    # |x| = max(-x, x); DVE tensor_scalar abs_max fails codegen
                    nc.vector.scalar_tensor_tensor(
                        out=A, in0=Acj, scalar=-1.0, in1=Acj,
                        op0=OP.mult, op1=OP.max,
                    )

                # --- G = 1.1*Aj - Ap ---
                if v["max_stage"] < 6:
                    continue
                if "g" in v["drop"]:
                    continue
                G = wrk.tile([P, F], bf16, tag="G")
                if v["g_mode"] == "act":
                    Aj11 = wrk.tile([P, F], bf16, tag="Aj11")
                    nc.scalar.activation(out=Aj11, in_=t["Aj"], func=AF.Copy,
                                         scale=1.1)
                    nc.vector.tensor_tensor(out=G, in0=Aj11, in1=t["Ap"],
                                            op=OP.subtract)
                else:
                    nc.vector.scalar_tensor_tensor(
                        out=G, in0=t["Aj"], scalar=1.1, in1=t["Ap"],
                        op0=OP.mult, op1=OP.subtract,
                    )

                # --- end-of-curve copies ---
                if v["max_stage"] < 7:
                    continue
                if "ends" in v["drop"]:
                    continue
                ends_eng = {"scalar": nc.scalar, "vector": nc.vector,
                            "gpsimd": nc.gpsimd}[v["ends_eng"]]
                for nm, dst in (("Ap", eAp), ("Aj", eAj), ("Ac", eAc)):
                    ends = t[nm].rearrange("p (j l) -> p j l", l=L)[:, :, L - 1 : L]
                    if v["ends_eng"] == "scalar":
                        ends_eng.copy(out=dst[:, cols], in_=ends)
                    else:
                        ends_eng.tensor_copy(out=dst[:, cols], in_=ends)

                # --- DVE segmented reduces ---
                if v["max_stage"] < 8:
                    continue
                if "reduces" in v["drop"]:
                    continue
                Acj3 = Acj.rearrange("p (j l) -> p j l", l=L)
                A3 = A.rearrange("p (j l) -> p j l", l=L)

                def segred(out_cols, src3, op):
                    lev = v["reduce_tree"]
                    if not lev:
                        nc.vector.tensor_reduce(out=out_cols, in_=src3, axis=AX.X, op=op)
                        return
                    cur3, w = src3, L
                    for Htile in (H1, H2)[:lev]:
                        h = w // 2
                        dst3 = Htile.rearrange("p (j l) -> p j l", l=h)[:, :J]
                        nc.vector.tensor_tensor(
                            out=dst3, in0=cur3[:, :, 0:h], in1=cur3[:, :, h:w], op=op)
                        cur3, w = dst3, h
                    nc.vector.tensor_reduce(out=out_cols, in_=cur3, axis=AX.X, op=op)

                segred(mnB[:, cols], A3, OP.min)
                segred(sAbs[:, cols], A3, OP.add)
                segred(sAcj[:, cols], Acj3, OP.add)

                # --- argmin-select: gint_c = sum (A==mn_c) * G ---
                if v["max_stage"] < 9:
                    continue
                if "sel" in v["drop"]:
                    continue
                if v["sel_mode"] == "bcast":
                    # mask = (A == broadcast(mn)); gint = sum(mask * G) per curve
                    mn3 = mnB[:, cols].rearrange("p j -> p j ()").broadcast_to([P, J, L])
                    gm = wrk.tile([P, F], bf16, tag="gm")
                    gm3 = gm.rearrange("p (j l) -> p j l", l=L)
                    nc.vector.tensor_tensor(out=gm3, in0=A3, in1=mn3, op=OP.is_equal)
                    nc.vector.tensor_tensor(out=A, in0=gm, in1=G, op=OP.mult)
                    segred(gint[:, cols], A3, OP.add)
                elif v["sel_mode"] == "scan":
                    # broadcast per-curve min along L via masked scan, then
                    # fused mask*G and one segmented reduce
                    nc.vector.tensor_copy(
                        out=zS.rearrange("p (j l) -> p j l", l=L)[:, :, 0:1],
                        in_=mnB[:, cols],
                    )
                    nc.vector.tensor_tensor_scan(
                        out=mnF, data0=Bt, data1=zS, initial=0.0,
                        op0=OP.mult, op1=OP.add,
                    )
                    nc.vector.tensor_tensor(out=gm, in0=A, in1=mnF, op=OP.is_equal)
                    nc.vector.tensor_tensor(out=A, in0=gm, in1=G, op=OP.mult)
                    nc.vector.tensor_reduce(
                        out=gint[:, cols], in_=A3, axis=AX.X, op=OP.add,
                    )
                else:
                    for j in range(J):
                        c = m * J + j
                        nc.vector.scalar_tensor_tensor(
                            out=junkD,
                            in0=A[:, j * L : (j + 1) * L],
                            scalar=mnB[:, c : c + 1],
                            in1=G[:, j * L : (j + 1) * L],
                            op0=OP.is_equal,
                            op1=OP.mult,
                            accum_out=gint[:, c : c + 1],
                        )

            # --- epilogue on [128, W] f32 column blocks ---
            if v["max_stage"] < 10 or "epi" in v["drop"]:
                if v["max_stage"] >= 10:
                    nc.vector.memset(accT, 0.0)
                nc.sync.dma_start(out=acc[:], in_=accT)
                return

            def epilogue_dve(lo, hi):
                W = hi - lo
                cs = slice(lo, hi)
                V = nc.vector
                # ls penalty
                V.tensor_tensor(out=t1[:, :W], in0=sAbs[:, cs], in1=sAcj[:, cs], op=OP.add)
                V.tensor_scalar(out=t1[:, :W], in0=t1[:, :W], scalar1=-0.5, scalar2=8.0,
                                op0=OP.mult, op1=OP.add)
                V.tensor_scalar(out=r1[:, :W], in0=t1[:, :W], scalar1=0.0, scalar2=None,
                                op0=OP.max)
                V.tensor_tensor(out=t2[:, :W], in0=sAbs[:, cs], in1=sAcj[:, cs], op=OP.subtract)
                V.tensor_scalar(out=t2[:, :W], in0=t2[:, :W], scalar1=-0.5, scalar2=8.0,
                                op0=OP.mult, op1=OP.add)
                V.tensor_scalar(out=r2[:, :W], in0=t2[:, :W], scalar1=0.0, scalar2=None,
                                op0=OP.max)
                V.tensor_tensor(out=t1[:, :W], in0=r1[:, :W], in1=r2[:, :W], op=OP.add)
                V.tensor_tensor(out=accT[:, LS0 + lo : LS0 + hi], in0=t1[:, :W],
                                in1=wT[:, cs], op=OP.mult)
                # crossover penalty
                V.tensor_scalar(out=t2[:, :W], in0=gint[:, cs], scalar1=3.0, scalar2=None,
                                op0=OP.mult)
                V.tensor_scalar(out=accT[:, P30 + lo : P30 + hi], in0=t2[:, :W],
                                scalar1=0.0, scalar2=None, op0=OP.max)
                # end-of-curve penalties
                V.tensor_tensor(out=t2[:, :W], in0=eAp[:, cs], in1=eAj[:, cs], op=OP.subtract)
                V.tensor_scalar(out=r1[:, :W], in0=t2[:, :W], scalar1=0.0, scalar2=None,
                                op0=OP.max)
                V.tensor_tensor(out=accT[:, E10 + lo : E10 + hi], in0=r1[:, :W],
                                in1=fT[:, cs], op=OP.mult)
                V.tensor_tensor(out=t2[:, :W], in0=eAj[:, cs], in1=eAc[:, cs], op=OP.subtract)
                V.tensor_scalar(out=accT[:, E20 + lo : E20 + hi], in0=t2[:, :W],
                                scalar1=0.0, scalar2=None, op0=OP.max)

            def epilogue(lo, hi):
                if v["epi_mode"] == "dve":
                    return epilogue_dve(lo, hi)
                W = hi - lo
                cs = slice(lo, hi)
                epi = nc.gpsimd if v["epi_on_pool"] else nc.vector
                epi.tensor_tensor(out=t1[:, :W], in0=sAbs[:, cs], in1=sAcj[:, cs], op=OP.add)
                nc.scalar.activation(out=r1[:, :W], in_=t1[:, :W], func=AF.Relu, scale=-0.5, bias=b8)
                epi.tensor_tensor(out=t2[:, :W], in0=sAbs[:, cs], in1=sAcj[:, cs], op=OP.subtract)
                nc.scalar.activation(out=r2[:, :W], in_=t2[:, :W], func=AF.Relu, scale=-0.5, bias=b8)
                epi.tensor_tensor(out=t1[:, :W], in0=r1[:, :W], in1=r2[:, :W], op=OP.add)
                epi.tensor_tensor(out=accT[:, LS0 + lo : LS0 + hi], in0=t1[:, :W],
                                  in1=wT[:, cs], op=OP.mult)
                nc.scalar.activation(out=accT[:, P30 + lo : P30 + hi], in_=gint[:, cs],
                                     func=AF.Relu, scale=3.0)
                epi.tensor_tensor(out=t2[:, :W], in0=eAp[:, cs], in1=eAj[:, cs], op=OP.subtract)
                nc.scalar.activation(out=r1[:, :W], in_=t2[:, :W], func=AF.Relu)
                epi.tensor_tensor(out=accT[:, E10 + lo : E10 + hi], in0=r1[:, :W],
                                  in1=fT[:, cs], op=OP.mult)
                epi.tensor_tensor(out=t2[:, :W], in0=eAj[:, cs], in1=eAc[:, cs], op=OP.subtract)
                nc.scalar.activation(out=accT[:, E20 + lo : E20 + hi], in_=t2[:, :W], func=AF.Relu)

            if v.get("chunked_epi"):
                for m in range(M):
                    epilogue(m * J, (m + 1) * J)
            else:
                epilogue(0, NCOL)

            nc.sync.dma_start(out=acc[:], in_=accT)

        if reps is None:
            body()
        else:
            with tc.For_i(0, reps, 1):
                body()

    nc.compile()
    return nc


_NC_CACHE = {}
LAST_RESULTS = None


def _get_nc(reps=None, variant=None):
    key = (reps, tuple(sorted((variant or {}).items())))
    if key not in _NC_CACHE:
        _NC_CACHE[key] = _build_kernel(reps, variant)
    return _NC_CACHE[key]


def _curve_layout(x_per_curve: np.ndarray) -> np.ndarray:
    """dev[p, m*J + j] corresponds to curve m*(P*J) + p*J + j."""
    return np.ascontiguousarray(
        x_per_curve.reshape(M, P, J).transpose(1, 0, 2).reshape(P, NCOL)
    )


def prep_in_maps(An_o, Ac_o, Aj_o, Ap_o, A_r, Ci, mask_lightresp):
    w_full = (mask_lightresp == 0).astype(np.float32)        # [C]
    Ci_end = np.ascontiguousarray(Ci[L - 1 :: L])            # [C]
    fit_full = ((Ci_end > FIT_AP_CI).astype(np.float32) * w_full)  # [C]

    in_maps = []
    for k in range(NCORES):
        cur = slice(k * S, (k + 1) * S)
        el = slice(k * NSH, (k + 1) * NSH)
        in_maps.append({
            "An": np.ascontiguousarray(An_o[el]),
            "Ar": np.ascontiguousarray(A_r[el]),
            "Ac": np.ascontiguousarray(Ac_o[el]),
            "Aj": np.ascontiguousarray(Aj_o[el]),
            "Ap": np.ascontiguousarray(Ap_o[el]),
            "wdev": _curve_layout(w_full[cur]),
            "fitw": _curve_layout(fit_full[cur]),
        })
    return in_maps


def kernel(An_o, Ac_o, Aj_o, Ap_o, A_r, Ci, Vcmax25, Jmax25, Rd25,
           dHa_Vcmax, dHa_Jmax, dHa_TPU, Topt_Vcmax, Topt_Jmax, Topt_TPU,
           mask_lightresp, variant=None):
    An_o, Ac_o, Aj_o, Ap_o, A_r, Ci = (
        np.asarray(x) for x in (An_o, Ac_o, Aj_o, Ap_o, A_r, Ci))
    (Vcmax25, Jmax25, Rd25, dHa_Vcmax, dHa_Jmax, dHa_TPU,
     Topt_Vcmax, Topt_Jmax, Topt_TPU, mask_lightresp) = (
        np.asarray(x) for x in (Vcmax25, Jmax25, Rd25, dHa_Vcmax, dHa_Jmax,
                                dHa_TPU, Topt_Vcmax, Topt_Jmax, Topt_TPU,
                                mask_lightresp))
    v = dict(VARIANT)
    if variant:
        v.update(variant)
    nc = _get_nc(variant=variant)
    in_maps = prep_in_maps(An_o, Ac_o, Aj_o, Ap_o, A_r, Ci, mask_lightresp)

    res = run_bass_kernel_spmd(nc, in_maps, core_ids=list(range(NCORES)))
    global LAST_RESULTS
    LAST_RESULTS = res
    blocks = [r["acc"].astype(np.float64) for r in res.results]

    if v["d_mode"] in ("gpsimd", "vector"):
        mse = sum(b[:, MSE0 : MSE0 + M].sum() for b in blocks)
    else:
        sa = sum(b[:, MSA0 : MSA0 + M].sum() for b in blocks)
        sb = sum(b[:, MSB0 : MSB0 + M].sum() for b in blocks)
        sc = sum(b[:, MSC0 : MSC0 + M].sum() for b in blocks)
        mse = sa + sb - 2.0 * sc
    apn = sum(b[:, APN0 : APN0 + M].sum() for b in blocks)
    if v["apn_eng"] != "scalar":
        apn = -apn
    p3 = sum(b[:, P30 : P30 + NCOL].sum() for b in blocks)
    ls = sum(b[:, LS0 : LS0 + NCOL].sum() for b in blocks)
    e1 = sum(b[:, E10 : E10 + NCOL].sum() for b in blocks)
    e2 = sum(b[:, E20 : E20 + NCOL].sum() for b in blocks)

    # host-side terms (tiny inputs only)
    w = (mask_lightresp == 0).astype(np.float64)
    x = Jmax25.astype(np.float64)
    y = Vcmax25.astype(np.float64)
    nw = w.sum()
    if nw > 0:
        my = (w * y).sum() / nw
        mx = (w * x).sum() / nw
        vy = (y - my) * w
        vx = (x - mx) * w
        denom = np.sqrt((vx * vx).sum()) * np.sqrt((vy * vy).sum())
        cost = (vx * vy).sum() / denom if denom != 0.0 else np.nan
    else:
        cost = np.nan
    if np.isnan(cost):
        cost = 0.0
    cost = min(cost, TARGET_R)

    relu = lambda z: np.maximum(z, 0.0)
    loss = mse * 10.0 / N
    loss += TARGET_R - cost
    loss += relu(-Rd25.astype(np.float64)).sum()
    loss += relu(-dHa_Vcmax.astype(np.float64)).sum() * 10.0
    loss += relu(-dHa_Jmax.astype(np.float64)).sum()
    loss += relu(-dHa_TPU.astype(np.float64)).sum()
    loss += relu(KELVIN - Topt_Vcmax.astype(np.float64)).sum()
    loss += relu(KELVIN - Topt_Jmax.astype(np.float64)).sum()
    loss += relu(KELVIN - Topt_TPU.astype(np.float64)).sum()
    loss += apn
    loss += e1 * 0.15
    loss += e2
    loss += p3
    loss += ls

    return np.asarray(loss, dtype=np.float32)
